# revision 14
# baseline (speedup 1.0000x reference)
"""Distributed Trainium2 Bass kernel for the quad-masked variance loss
(nn_Cons_Loss_79027398246842), SPMD across 8 NeuronCores.

Math: the quads are axis-aligned rectangles, so the point-in-polygon mask
separates into row_mask[q,h] * col_mask[q,w].  The gt>0 gate g is folded
into the summand host-side (exact: g in {0,1} so s1 = sum mask*g*p,
s2 = sum mask*(g*p)^2, cnt = sum mask*g).  With s1/s2/cnt the masked sums
per quad, the loss is
    sum_{l,q} where(cnt>0, (s2 - 2*mean*s1 + mean^2*cnt)/max(cnt,1), 0),
    mean = s1/max(cnt,1).

Sharding: W (columns) split across the 8 cores (64 columns each).  Each
core computes partial (s1[l,q], s2[l,q], cnt[q]) over its columns for ALL
64 quads via a two-stage contraction:
  stage 1 (TensorE, bf16): contract H in 4 chunks of 128 rows with the
    transposed row mask as the stationary operand.  Each chunk issues TWO
    N=288 matmuls, one per 32-column half, with the second landing on
    PSUM partitions 64..127 (tile_position via AP base partitions) -- so
    the [quad, channel, col] accumulator occupies all 128 partitions and
    one PSUM bank.
  stage 2 (VectorE): multiply by the (half-split) column mask and reduce
    over the 32 columns -- half the per-partition elements of a 64-wide
    layout.
The per-core [128, 9] partials (quad q cols 0:32 on partition q, cols
32:64 on partition 64+q) are summed host-side.

Input marshalling (host, part of shard prep): pg[h, c, ch, w] bf16 with
channels [g*pred x4, g]; squares (g*p)^2 are computed on device.

Engine plan per core:
  scalar : pg chunk 0+1 DMAs, all four squares (in DMA-arrival order),
           out DMA
  sync   : aux DMA, pg chunk 2 DMA
  gpsimd : pg chunk 3 DMA (software DGE = third parallel queue), col
           compares + AND (half-split layout)
  vector : row compares + AND per half, stage-2 colM multiply + reduce
  tensor : per chunk, two N=288 matmuls (col halves)

Semaphore ledger:
  sQ: t1a01=1 t2a01=2 t1a23=3 t2a23=4   (vector row compares)
  sR: rta01=1 rta23=2                   (vector row-mask AND)
  sS: sq0=1 sq1=2 sq2=3 sq3=4           (scalar squares)
  sX: c1=1 c2=2    sC: colM=1           (gpsimd col mask)
  sV: M=1 red=2                         (vector stage 2)
  sT: last-mm=1
  dA/dP0..3/dO: DMA completions (+16 each)
"""
import numpy as np
from contextlib import ExitStack

from concourse import bacc, bass
import concourse.mybir as mybir

F32 = mybir.dt.float32
BF16 = mybir.dt.bfloat16
ALU = mybir.AluOpType
ACT = mybir.ActivationFunctionType

N_CORES = 8
L, H, W = 4, 512, 512
NB = 64
WL = W // N_CORES          # 64 columns per core
WLH = WL // 2              # 32-column half (stage-2 partition doubling)
HC = 128                   # h-chunk (partition dim)
NCH = H // HC              # 4 chunks
NT = 2 * L + 1             # 9 channels: [g*p x4, g, (g*p)^2 x4]
PGC = L + 1                # channels shipped from host: [g*p x4, g]
EPS = 1e-5

# aux2 input layout [128, 200] f32 (host-prepared constants):
#   [:, 0:64]    lo row broadcast (row-mask lower bound per quad)
#   [:, 64:128]  hi row broadcast
#   [:, 128]     x0 - WL*core, quad q on partitions q and 64+q
#   [:, 129]     x1 - WL*core, likewise
#   [:, 130:134] pycol[p, c] = 128*c + p
#   [:, 136:168] px half-grid: partition q -> 0..31, partition 64+q -> 32..63
AUX2_W = 168


def build_kernel():
    nc = bacc.Bacc("TRN2", target_bir_lowering=False, debug=False,
                   enable_asserts=False)

    pg_e = nc.dram_tensor("pg", [HC, NCH, PGC, WL], BF16, kind="ExternalInput")
    aux_e = nc.dram_tensor("aux2", [HC, AUX2_W], F32, kind="ExternalInput")
    out_e = nc.dram_tensor("out", [HC, NT], F32, kind="ExternalOutput")

    ctx = ExitStack()
    sem = lambda name: ctx.enter_context(nc.semaphore(name))
    sb = lambda name, shape, dt=F32: ctx.enter_context(
        nc.sbuf_tensor(name, shape, dt))
    ps = lambda name, shape: ctx.enter_context(
        nc.psum_tensor(name, shape, F32))

    with ctx:
        dA = sem("dA"); dO = sem("dO")
        dPs = [sem(f"dP{c}") for c in range(NCH)]
        sQ = sem("sQ"); sR = sem("sR"); sV = sem("sV"); sS = sem("sS")
        sX = sem("sX"); sC = sem("sC"); sT = sem("sT")

        AX = sb("AX", [HC, AUX2_W])
        # PA channels: 0:L = g*p (DMA), L = g (DMA), L+1:NT = (g*p)^2 (sq)
        PA = sb("PA", [HC, NCH, NT, WL], BF16)
        t1a = sb("t1a", [HC, NCH, NB], BF16)
        t2a = sb("t2a", [HC, NCH, NB], BF16)
        rta = sb("rta", [HC, NCH, NB], BF16)
        c1 = sb("c1", [HC, WLH])
        c2 = sb("c2", [HC, WLH])
        colM = sb("colM", [HC, WLH])
        M = sb("M", [HC, NT, WLH], BF16)
        partial = sb("partial", [HC, NT])

        # one PSUM bank: quad q / cols 0:32 on partition q, cols 32:64 on
        # partition 64+q
        D = ps("D", [HC, NT, WLH])

        lo_b = AX[:, 0:NB]
        hi_b = AX[:, NB:2 * NB]
        x0p = AX[:, 128:129]
        x1p = AX[:, 129:130]
        px_b = AX[:, 136:136 + WLH]

        # Hand-rolled Block: same per-engine body wiring as nc.Block()
        # but WITHOUT the exit all-engine barrier -- the module-level
        # $S[2] butterfly + semaphore-clear epilogue that follows already
        # synchronizes all engines, so the extra drain+butterfly layer
        # (~0.8us) is redundant.
        _blk = "blk"
        _end_bb = _blk + "_end"
        _last_body = {}

        def _on(engine_obj, f):
            body = f"{_blk}_{engine_obj.engine.value}_{nc.next_id()}"
            if engine_obj not in _last_body:
                engine_obj.br(body)
            else:
                with nc.body(_last_body[engine_obj]):
                    engine_obj.br(body)
            _last_body[engine_obj] = body
            with nc.body(body):
                f(engine_obj)

        class _Blk:
            scalar = staticmethod(lambda f: _on(nc.scalar, f))
            sync = staticmethod(lambda f: _on(nc.sync, f))
            vector = staticmethod(lambda f: _on(nc.vector, f))
            gpsimd = staticmethod(lambda f: _on(nc.gpsimd, f))
            tensor = staticmethod(lambda f: _on(nc.tensor, f))

        block = _Blk()
        if True:

            @block.scalar
            def _(scalar):
                for c in (0, 2):
                    scalar.dma_start(
                        out=PA[:, c, 0:PGC, :], in_=pg_e[:, c, :, :]
                    ).then_inc(dPs[c], 16)
                for c in (0, 2, 3):          # DMA-arrival order; sq1 on vector
                    scalar.wait_ge(dPs[c], 16)
                    scalar.activation(
                        out=PA[:, c, L + 1:NT, :], in_=PA[:, c, 0:L, :],
                        func=ACT.Square,
                    ).then_inc(sS)               # sS=1..3 (sq0,sq2,sq3)

            @block.sync
            def _(sync):
                sync.dma_start(out=AX[:, :], in_=aux_e[:, :]).then_inc(dA, 16)
                sync.dma_start(
                    out=PA[:, 1, 0:PGC, :], in_=pg_e[:, 1, :, :]
                ).then_inc(dPs[1], 16)
                # out DMA from the long-idle sync engine: its branch/drain
                # cost after the issue is far cheaper than scalar's
                sync.wait_ge(sV, 3)
                sync.dma_start(out=out_e[:, :], in_=partial[:, :]).then_inc(
                    dO, 16)

            @block.vector
            def _(vector):
                vector.wait_ge(dA, 16)

                def row_half(h, sq_base):
                    cs = slice(2 * h, 2 * h + 2)
                    lo2 = lo_b.unsqueeze(1).broadcast_to((HC, 2, NB))
                    hi2 = hi_b.unsqueeze(1).broadcast_to((HC, 2, NB))
                    py2 = AX[:, 130 + 2 * h:132 + 2 * h].unsqueeze(
                        2).broadcast_to((HC, 2, NB))
                    vector.tensor_tensor(
                        out=t1a[:, cs, :], in0=lo2, in1=py2, op=ALU.is_le,
                    ).then_inc(sQ)
                    vector.tensor_tensor(
                        out=t2a[:, cs, :], in0=hi2, in1=py2, op=ALU.is_ge,
                    ).then_inc(sQ)
                    vector.wait_ge(sQ, sq_base)      # self-sem: RAW on t1a/t2a
                    vector.tensor_tensor(
                        out=rta[:, cs, :], in0=t1a[:, cs, :],
                        in1=t2a[:, cs, :], op=ALU.mult,
                    ).then_inc(sR)

                row_half(0, 2)                       # sQ=1,2  sR=1
                row_half(1, 4)                       # sQ=3,4  sR=2
                vector.wait_ge(dPs[1], 16)
                vector.tensor_tensor(
                    out=PA[:, 1, L + 1:NT, :], in0=PA[:, 1, 0:L, :],
                    in1=PA[:, 1, 0:L, :], op=ALU.mult,
                ).then_inc(sV)                       # sV=1 (sq1)

                # stage 2: colM multiply + w-reduce over all 9 channels
                vector.wait_ge(sT, 1)
                vector.wait_ge(sC, 1)
                col_b = colM[:, :].unsqueeze(1).broadcast_to((HC, NT, WLH))
                vector.tensor_tensor(
                    out=M[:, :, :], in0=D[:, :, :], in1=col_b, op=ALU.mult,
                ).then_inc(sV)                       # sV=2
                # self-sem: orders the M reads below after the writes land
                vector.wait_ge(sV, 2)
                vector.tensor_reduce(
                    out=partial[:, :], in_=M[:, :, :],
                    axis=mybir.AxisListType.X, op=ALU.add,
                ).then_inc(sV)                       # sV=3

            @block.gpsimd
            def _(gpsimd):
                gpsimd.dma_start(
                    out=PA[:, 3, 0:PGC, :], in_=pg_e[:, 3, :, :]
                ).then_inc(dPs[3], 16)
                gpsimd.wait_ge(dA, 16)
                gpsimd.tensor_scalar(
                    out=c1[:, :], in0=px_b, scalar1=x0p,
                    scalar2=None, op0=ALU.is_ge,
                ).then_inc(sX)                       # sX=1
                gpsimd.tensor_scalar(
                    out=c2[:, :], in0=px_b, scalar1=x1p,
                    scalar2=None, op0=ALU.is_le,
                ).then_inc(sX)                       # sX=2
                gpsimd.wait_ge(sX, 2)                # self-sem: RAW on c1/c2
                gpsimd.tensor_tensor(
                    out=colM[:, :], in0=c1[:, :], in1=c2[:, :], op=ALU.mult,
                ).then_inc(sC)                       # sC=1

            @block.tensor
            def _(tensor):
                # chunks in DMA-arrival order; deps: rta halves (sR),
                # squares (sS, also ordered 0,3,2,1)
                order = (0, 2, 3, 1)
                chunk_waits = {0: [(sR, 1), (sS, 1)], 2: [(sR, 2), (sS, 2)],
                               3: [(sS, 3)], 1: [(sV, 1)]}
                for i, c in enumerate(order):
                    for s, v in chunk_waits[c]:
                        tensor.wait_ge(s, v)
                    st = dict(start=(i == 0), stop=(i == NCH - 1))
                    tensor.matmul(
                        D[0:NB, :, :], rta[:, c, :],
                        PA[:, c, :, 0:WLH], **st)
                    mm = tensor.matmul(
                        D[NB:HC, :, :], rta[:, c, :],
                        PA[:, c, :, WLH:WL], **st)
                    if i == NCH - 1:
                        mm.then_inc(sT)              # sT=1

            for _eng, _lb in _last_body.items():
                with nc.body(_lb, parent=nc.cur_bb,
                             allow_existing_parent=True):
                    _eng.br(_end_bb)
            nc.switch_bb(_end_bb)

    nc.compile()
    return nc


_NC = None


def _get_nc():
    global _NC
    if _NC is None:
        _NC = build_kernel()
    return _NC


def _make_aux(boxes, core):
    aux2 = np.zeros((HC, AUX2_W), dtype=np.float32)
    eps_q = np.float32(2.0 * EPS) / (boxes[:, 2] - boxes[:, 0])
    aux2[:, 0:NB] = boxes[:, 1] + eps_q          # lo row, all partitions
    aux2[:, NB:2 * NB] = boxes[:, 5] - eps_q     # hi row
    x0 = boxes[:, 0] - WL * core                 # core-local col bounds
    x1 = boxes[:, 2] - WL * core
    aux2[0:NB, 128] = x0; aux2[NB:HC, 128] = x0
    aux2[0:NB, 129] = x1; aux2[NB:HC, 129] = x1
    aux2[:, 130:134] = (
        np.arange(H, dtype=np.float32).reshape(NCH, HC).T)  # pycol
    px = np.arange(WLH, dtype=np.float32)[None, :]
    aux2[0:NB, 136:136 + WLH] = px               # cols 0:32 for quad q
    aux2[NB:HC, 136:136 + WLH] = px + WLH        # cols 32:64 for quad q
    return aux2


def make_in_maps(pred, gt, boxes):
    bf16 = mybir.dt.np(BF16)
    pred = np.asarray(pred, dtype=np.float32)
    gt = np.asarray(gt, dtype=np.float32)
    boxes = np.asarray(boxes, dtype=np.float32).reshape(NB, 8)
    # fold the gt>0 gate into pred (exact: g in {0,1}) and pack
    # [g*p x4, g] -> [HC, NCH, PGC, W] bf16, h-within-chunk on partitions
    g = (gt[0] > 0).astype(np.float32)           # [H, W]
    pg = np.empty((HC, NCH, PGC, W), dtype=bf16)
    pg[:, :, 0:L, :] = (pred[0] * g[None]).reshape(L, NCH, HC, W).transpose(
        2, 1, 0, 3).astype(bf16)
    pg[:, :, L, :] = g.reshape(NCH, HC, W).transpose(1, 0, 2).astype(bf16)
    in_maps = []
    for i in range(N_CORES):
        ws = slice(WL * i, WL * (i + 1))
        in_maps.append({
            "pg": np.ascontiguousarray(pg[:, :, :, ws]),
            "aux2": _make_aux(boxes, i),
        })
    return in_maps


def finish(partials):
    """Host-side unshard: sum per-core partials and apply the loss formula."""
    tot = np.sum(np.stack(partials, 0), axis=0)  # [128, 9]
    tot = tot[0:NB] + tot[NB:HC]                 # combine column halves
    s1 = tot[:, 0:L].T        # [L, NB]
    cnt = tot[:, L]
    s2 = tot[:, L + 1:NT].T
    safe = np.maximum(cnt, 1.0)
    mean = s1 / safe[None, :]
    per = (s2 - 2.0 * mean * s1 + mean * mean * cnt[None, :]) / safe[None, :]
    per = np.where(cnt[None, :] > 0, per, 0.0)
    return np.float32(per.sum(dtype=np.float32))


def kernel(pred, gt, boxes):
    from concourse.bass_utils import run_bass_kernel_spmd

    nc = _get_nc()
    in_maps = make_in_maps(pred, gt, boxes)
    res = run_bass_kernel_spmd(nc, in_maps, core_ids=list(range(N_CORES)))
    return finish([r["out"] for r in res.results])


if __name__ == "__main__":
    build_kernel()
    print("build + compile OK")
